# revision 13
# baseline (speedup 1.0000x reference)
"""Trainium2 Bass kernel for nn_BaseObservationModel (topk masking).

Computes, for x = (32,1024,2048) inputs flattened to rows of D=2048:
    noisy    = data + 0.1*noise
    mask     = positions of the 512 largest rand_vals per row
    masked   = noisy * (1-mask);  mask_inverse = (1-mask) as f32

Device algorithm (per row, exact):
  rand_vals are j*2^-23 (jax uniform) -> probe thresholds on the odd 2^-24
  grid never collide with data. Regula-falsi bracket search (6 counting
  probes, counts via ACT Sign+accum / DVE is_gt+accum) finds hi with
  c_hi = #{r > hi} in [496, 511]. Then w = r*(r<=hi), top-16 of w via
  DVE Max8 + MatchReplace + Max8, and t* = the (512-c_hi)-th largest of w
  == the 512th largest of the row. mask_inverse = (r < t*).
  Rows where the 512th value is tied at the boundary (2 rows in this
  dataset) are patched exactly on the host from the device output.

Data parallel: 32768 rows sharded 4096/core across 8 cores.
"""

import numpy as np

# ---------------- hardcoded problem config ----------------
B_SHAPE = (32, 1024, 2048)
D = 2048
K = 512
N_CORES = 8
ROWS_TOTAL = 32768
ROWS_PER_CORE = ROWS_TOTAL // N_CORES  # 4096
P = 128
N_TILES = ROWS_PER_CORE // P  # 32
GROUP = 4  # tiles per probe-batch group
TARGETS = [512.0, 512.0, 508.0, 507.0, 506.0, 505.0]
# engine per probe round: 'act' = Sign+accum on ScalarE, 'dve' = is_gt on VectorE
ROUND_ENGINES = ["act", "act", "act", "act", "act", "dve"]

LO0 = 3.0 / 16777216.0
CLO0 = 2048.0
HI0 = 16777215.0 / 16777216.0
CHI0 = 0.0
MAGIC = 12582912.0
P23 = 8388608.0
IP23 = 1.0 / 8388608.0
NOISE_STD = 0.1

_CACHE = {}


def emit(tc, nc, r_d, d_d, n_d, om_d, omi_d, n_tiles, group, ctx, loop_n=1):
    """Emit the tile program. r_d/d_d/n_d inputs, om_d/omi_d outputs: DRAM
    tensors of [n_tiles*128, 2048] f32."""
    from concourse import mybir
    from concourse.alu_op_type import AluOpType as AO

    dt = mybir.dt.float32
    AF = mybir.ActivationFunctionType
    AX = mybir.AxisListType

    G = group
    n_groups = (n_tiles + G - 1) // G

    rp = ctx.enter_context(tc.tile_pool(name="rp", bufs=2))
    dp = ctx.enter_context(tc.tile_pool(name="dp", bufs=3))
    np_ = ctx.enter_context(tc.tile_pool(name="np", bufs=3))
    scr = ctx.enter_context(tc.tile_pool(name="scr", bufs=2))
    wp01 = ctx.enter_context(tc.tile_pool(name="wp01", bufs=2))
    wp = ctx.enter_context(tc.tile_pool(name="wp", bufs=2))
    mip = ctx.enter_context(tc.tile_pool(name="mip", bufs=2))
    smp = ctx.enter_context(tc.tile_pool(name="smp", bufs=2))  # small per-group state
    cst = ctx.enter_context(tc.tile_pool(name="cst", bufs=1))

    # constants
    iota16 = cst.tile([P, 16], dt, tag="iota16", name="iota16")
    nc.gpsimd.iota(
        iota16[:],
        pattern=[[1, 16]],
        base=1,
        channel_multiplier=0,
        allow_small_or_imprecise_dtypes=True,
    )

    for _rep in range(loop_n):
        _emit_groups(tc, nc, r_d, d_d, n_d, om_d, omi_d, n_tiles, G, n_groups,
                     iota16, rp, dp, np_, scr, wp01, wp, mip, smp)


def _emit_groups(tc, nc, r_d, d_d, n_d, om_d, omi_d, n_tiles, G, n_groups,
                 iota16, rp, dp, np_, scr, wp01, wp, mip, smp):
    from concourse import mybir
    from concourse.alu_op_type import AluOpType as AO

    dt = mybir.dt.float32
    AF = mybir.ActivationFunctionType
    AX = mybir.AxisListType

    for g in range(n_groups):
        tiles = [g * G + i for i in range(G) if g * G + i < n_tiles]
        Gg = len(tiles)

        # ---- load rand tiles for the group ----
        r_t = []
        for i, t in enumerate(tiles):
            rt = rp.tile([P, D], dt, tag=f"r{i}", name=f"r{i}")
            nc.sync.dma_start(rt[:], r_d[t * P : (t + 1) * P, :])
            r_t.append(rt)

        # ---- per-group state [P, Gg] ----
        def st(tag):
            return smp.tile([P, Gg], dt, tag=tag, name=tag)

        LO, CLO, HI, CHI = st("LO"), st("CLO"), st("HI"), st("CHI")
        T, NT, SR, C = st("T"), st("NT"), st("SR"), st("C")
        UP, DN, A, RPc = st("UP"), st("DN"), st("A"), st("RP")
        M, TST = st("M"), st("TST")

        nc.vector.memset(LO[:], LO0)
        nc.vector.memset(CLO[:], CLO0)
        nc.vector.memset(HI[:], HI0)
        nc.vector.memset(CHI[:], CHI0)

        for rnd, (tgt, eng) in enumerate(zip(TARGETS, ROUND_ENGINES)):
            # T = LO + round_even((HI-LO)*clip((CLO-tgt)/(CLO-CHI)))
            nc.vector.tensor_tensor(A[:], CLO[:], CHI[:], AO.subtract)
            nc.vector.reciprocal(RPc[:], A[:])
            nc.vector.tensor_scalar(A[:], CLO[:], float(tgt), None, AO.subtract)
            nc.vector.tensor_tensor(A[:], A[:], RPc[:], AO.mult)
            nc.vector.tensor_scalar(A[:], A[:], 0.02, 0.98, AO.max, AO.min)
            nc.vector.tensor_tensor(T[:], HI[:], LO[:], AO.subtract)
            nc.vector.tensor_tensor(A[:], T[:], A[:], AO.mult)
            nc.vector.tensor_scalar(A[:], A[:], P23, MAGIC, AO.mult, AO.add)
            nc.vector.tensor_scalar(A[:], A[:], MAGIC, None, AO.subtract)
            nc.vector.tensor_scalar(A[:], A[:], IP23, None, AO.mult)
            nc.vector.tensor_tensor(T[:], LO[:], A[:], AO.add)

            if eng == "act":
                # NT on ACT so probe activations need no cross-engine wait
                nc.scalar.mul(NT[:], T[:], -1.0)
                for i in range(Gg):
                    sgn = scr.tile([P, D], dt, tag="sgn", name="sgn")
                    nc.scalar.activation(
                        sgn[:],
                        r_t[i][:],
                        AF.Sign,
                        bias=NT[:, i : i + 1],
                        scale=1.0,
                        accum_out=SR[:, i : i + 1],
                    )
                nc.vector.tensor_scalar(C[:], SR[:], 2048.0, 0.5, AO.add, AO.mult)
            else:
                for i in range(Gg):
                    sgn = scr.tile([P, D], dt, tag="sgn", name="sgn")
                    nc.vector.tensor_scalar(
                        sgn[:],
                        r_t[i][:],
                        T[:, i : i + 1],
                        None,
                        AO.is_gt,
                        AO.add,
                        accum_out=C[:, i : i + 1],
                    )

            nc.vector.tensor_scalar(UP[:], C[:], 512.0, None, AO.is_ge)
            nc.vector.tensor_scalar(DN[:], C[:], 511.0, None, AO.is_le)
            for dst, src, sel in (
                (LO, T, UP),
                (CLO, C, UP),
                (HI, T, DN),
                (CHI, C, DN),
            ):
                nc.vector.tensor_tensor(A[:], src[:], dst[:], AO.subtract)
                nc.vector.tensor_tensor(A[:], A[:], sel[:], AO.mult)
                nc.vector.tensor_tensor(dst[:], dst[:], A[:], AO.add)

        # m = clip(512 - CHI, 1, 16)
        nc.vector.tensor_scalar(M[:], CHI[:], -1.0, 512.0, AO.mult, AO.add)
        nc.vector.tensor_scalar(M[:], M[:], 1.0, 16.0, AO.max, AO.min)

        # ---- apply phase ----
        for i, t in enumerate(tiles):
            row = t * P
            dtile = dp.tile([P, D], dt, tag="d", name="dtl")
            ntile = np_.tile([P, D], dt, tag="n", name="ntl")
            nc.sync.dma_start(dtile[:], d_d[row : row + P, :])
            nc.sync.dma_start(ntile[:], n_d[row : row + P, :])
            # noisy = data + 0.1*noise  (scale on ACT, add on GPSIMD)
            nc.scalar.activation(ntile[:], ntile[:], AF.Copy, bias=0.0, scale=NOISE_STD)
            nc.gpsimd.tensor_tensor(dtile[:], dtile[:], ntile[:], AO.add)

            # w = r * (r <= hi)
            w01 = wp01.tile([P, D], dt, tag="w01", name="w01")
            nc.gpsimd.tensor_scalar(
                w01[:], r_t[i][:], HI[:, i : i + 1], None, AO.is_le
            )
            w = wp.tile([P, D], dt, tag="w", name="w")
            nc.gpsimd.tensor_tensor(w[:], r_t[i][:], w01[:], AO.mult)

            # top16 of w
            t16 = smp.tile([P, 16], dt, tag="t16", name="t16")
            nc.vector.max(t16[:, 0:8], w[:])
            w2 = scr.tile([P, D], dt, tag="w2", name="w2")
            nc.vector.match_replace(w2[:], t16[:, 0:8], w[:], 0.0)
            nc.vector.max(t16[:, 8:16], w2[:])

            # t* = t16[m-1] : onehot(iota16 == m) dot t16
            oh = smp.tile([P, 16], dt, tag="oh", name="oh")
            nc.vector.tensor_scalar(
                oh[:], iota16[:], M[:, i : i + 1], None, AO.is_equal
            )
            nc.vector.tensor_tensor(oh[:], oh[:], t16[:], AO.mult)
            nc.vector.tensor_reduce(TST[:, i : i + 1], oh[:], AX.X, AO.add)

            # mask_inverse = (r < t*)
            mi = mip.tile([P, D], dt, tag="mi", name="mi")
            nc.gpsimd.tensor_scalar(
                mi[:], r_t[i][:], TST[:, i : i + 1], None, AO.is_lt
            )
            # masked = noisy * mask_inverse
            nc.vector.tensor_tensor(dtile[:], dtile[:], mi[:], AO.mult)

            nc.sync.dma_start(om_d[row : row + P, :], dtile[:])
            nc.sync.dma_start(omi_d[row : row + P, :], mi[:])


def build_program(n_tiles=N_TILES, group=GROUP, loop_n=1):
    """Build the SPMD bass program (one core's view)."""
    from contextlib import ExitStack

    import concourse.bacc as bacc
    import concourse.tile as tile
    from concourse import mybir

    rows = n_tiles * P
    nc = bacc.Bacc(None, debug=False)
    dt = mybir.dt.float32
    r_d = nc.dram_tensor("rand", [rows, D], dt, kind="ExternalInput")
    d_d = nc.dram_tensor("data", [rows, D], dt, kind="ExternalInput")
    n_d = nc.dram_tensor("noise", [rows, D], dt, kind="ExternalInput")
    om_d = nc.dram_tensor("masked", [rows, D], dt, kind="ExternalOutput")
    omi_d = nc.dram_tensor("maskinv", [rows, D], dt, kind="ExternalOutput")
    with tile.TileContext(nc) as tc, ExitStack() as ctx:
        emit(tc, nc, r_d, d_d, n_d, om_d, omi_d, n_tiles, group, ctx, loop_n=loop_n)
    return nc


def _tie_patch(r, mask_inv, masked, noisy_fn):
    """Exactly fix rows where the 512th value is tied at the boundary.
    jax top_k keeps the lowest-index elements among tied values."""
    rowsum = mask_inv.sum(axis=1)
    bad = np.where(rowsum != np.float32(D - K))[0]
    for row in bad:
        rr = r[row]
        mask = mask_inv[row] == 0.0
        if not mask.any():
            continue
        tstar = rr[mask].min()
        n_gt = int((rr > tstar).sum())
        need = K - n_gt
        tie_idx = np.where(rr == tstar)[0]
        if need < 0 or need > len(tie_idx):
            continue  # not a tie artifact; leave for the caller's check
        keep = tie_idx[need:]
        if len(keep):
            nz = noisy_fn(row)
            mask_inv[row, keep] = 1.0
            masked[row, keep] = nz[keep]
    return mask_inv, masked


def kernel(data, noise, rand_vals):
    from concourse.bass_utils import run_bass_kernel_spmd

    if "nc" not in _CACHE:
        nc = build_program()
        if not nc.is_finalized():
            nc.finalize()
        _CACHE["nc"] = nc
    nc = _CACHE["nc"]

    d2 = np.ascontiguousarray(data.reshape(ROWS_TOTAL, D), dtype=np.float32)
    n2 = np.ascontiguousarray(noise.reshape(ROWS_TOTAL, D), dtype=np.float32)
    r2 = np.ascontiguousarray(rand_vals.reshape(ROWS_TOTAL, D), dtype=np.float32)

    in_maps = []
    for c in range(N_CORES):
        s = slice(c * ROWS_PER_CORE, (c + 1) * ROWS_PER_CORE)
        in_maps.append(
            {
                "rand": np.ascontiguousarray(r2[s]),
                "data": np.ascontiguousarray(d2[s]),
                "noise": np.ascontiguousarray(n2[s]),
            }
        )

    res = run_bass_kernel_spmd(nc, in_maps, list(range(N_CORES)))
    _CACHE["last_results"] = res
    masked = np.concatenate([res.results[c]["masked"] for c in range(N_CORES)], axis=0)
    mask_inv = np.concatenate(
        [res.results[c]["maskinv"] for c in range(N_CORES)], axis=0
    )

    def noisy_fn(row):
        return (d2[row] + np.float32(NOISE_STD) * n2[row]).astype(np.float32)

    mask_inv, masked = _tie_patch(r2, mask_inv, masked, noisy_fn)

    return masked.reshape(B_SHAPE), mask_inv.reshape(B_SHAPE)


# revision 15
# speedup vs baseline: 1.5625x; 1.5625x over previous
"""Trainium2 Bass kernel for nn_BaseObservationModel (topk masking).

Computes, for x = (32,1024,2048) inputs flattened to rows of D=2048:
    noisy    = data + 0.1*noise
    mask     = positions of the 512 largest rand_vals per row
    masked   = noisy * (1-mask);  mask_inverse = (1-mask) as f32

Device algorithm (per row, exact):
  rand_vals are j*2^-23 (jax uniform) -> probe thresholds on the odd 2^-24
  grid never collide with data. Regula-falsi bracket search (6 counting
  probes, counts via ACT Sign+accum / DVE is_gt+accum) finds hi with
  c_hi = #{r > hi} in [496, 511]. Then w = r*(r<=hi), top-16 of w via
  DVE Max8 + MatchReplace + Max8, and t* = the (512-c_hi)-th largest of w
  == the 512th largest of the row. mask_inverse = (r < t*).
  Rows where the 512th value is tied at the boundary (2 rows in this
  dataset) are patched exactly on the host from the device output.

Data parallel: 32768 rows sharded 4096/core across 8 cores.
"""

import numpy as np

# ---------------- hardcoded problem config ----------------
B_SHAPE = (32, 1024, 2048)
D = 2048
K = 512
N_CORES = 8
ROWS_TOTAL = 32768
ROWS_PER_CORE = ROWS_TOTAL // N_CORES  # 4096
P = 128
N_TILES = ROWS_PER_CORE // P  # 32
GROUP = 4  # tiles per probe-batch group
TARGETS = [512.0, 512.0, 508.0, 507.0, 506.0, 505.0]
# engine per probe round: 'act' = Sign+accum on ScalarE, 'dve' = is_gt on VectorE
ROUND_ENGINES = ["act", "act", "act", "act", "act", "dve"]

LO0 = 3.0 / 16777216.0
CLO0 = 2048.0
HI0 = 16777215.0 / 16777216.0
CHI0 = 0.0
MAGIC = 12582912.0
P23 = 8388608.0
IP23 = 1.0 / 8388608.0
NOISE_STD = 0.1

USE_GPSIMD = True  # route elementwise apply-phase ops to GPSIMD vs DVE

_CACHE = {}


def emit(tc, nc, r_d, d_d, n_d, om_d, omi_d, n_tiles, group, ctx, loop_n=1):
    """Emit the tile program. r_d/d_d/n_d inputs, om_d/omi_d outputs: DRAM
    tensors of [n_tiles*128, 2048] f32."""
    from concourse import mybir
    from concourse.alu_op_type import AluOpType as AO

    dt = mybir.dt.float32
    AF = mybir.ActivationFunctionType
    AX = mybir.AxisListType

    G = group
    n_groups = (n_tiles + G - 1) // G

    rp = ctx.enter_context(tc.tile_pool(name="rp", bufs=2))
    dp = ctx.enter_context(tc.tile_pool(name="dp", bufs=3))
    np_ = ctx.enter_context(tc.tile_pool(name="np", bufs=3))
    scr = ctx.enter_context(tc.tile_pool(name="scr", bufs=2))
    wp01 = ctx.enter_context(tc.tile_pool(name="wp01", bufs=2))
    wp = ctx.enter_context(tc.tile_pool(name="wp", bufs=2))
    mip = ctx.enter_context(tc.tile_pool(name="mip", bufs=2))
    smp = ctx.enter_context(tc.tile_pool(name="smp", bufs=2))  # small per-group state
    cst = ctx.enter_context(tc.tile_pool(name="cst", bufs=1))

    # constants
    iota16 = cst.tile([P, 16], dt, tag="iota16", name="iota16")
    nc.gpsimd.iota(
        iota16[:],
        pattern=[[1, 16]],
        base=1,
        channel_multiplier=0,
        allow_small_or_imprecise_dtypes=True,
    )

    for _rep in range(loop_n):
        _emit_groups(tc, nc, r_d, d_d, n_d, om_d, omi_d, n_tiles, G, n_groups,
                     iota16, rp, dp, np_, scr, wp01, wp, mip, smp)


def _emit_groups(tc, nc, r_d, d_d, n_d, om_d, omi_d, n_tiles, G, n_groups,
                 iota16, rp, dp, np_, scr, wp01, wp, mip, smp):
    from concourse import mybir
    from concourse.alu_op_type import AluOpType as AO

    dt = mybir.dt.float32
    AF = mybir.ActivationFunctionType
    AX = mybir.AxisListType

    for g in range(n_groups):
        tiles = [g * G + i for i in range(G) if g * G + i < n_tiles]
        Gg = len(tiles)

        # ---- load rand tiles for the group ----
        r_t = []
        for i, t in enumerate(tiles):
            rt = rp.tile([P, D], dt, tag=f"r{i}", name=f"r{i}")
            nc.sync.dma_start(rt[:], r_d[t * P : (t + 1) * P, :])
            r_t.append(rt)

        # ---- per-group state [P, Gg] ----
        def st(tag):
            return smp.tile([P, Gg], dt, tag=tag, name=tag)

        LO, CLO, HI, CHI = st("LO"), st("CLO"), st("HI"), st("CHI")
        T, NT, SR, C = st("T"), st("NT"), st("SR"), st("C")
        UP, DN, A, RPc = st("UP"), st("DN"), st("A"), st("RP")
        M, TST = st("M"), st("TST")

        nc.vector.memset(LO[:], LO0)
        nc.vector.memset(CLO[:], CLO0)
        nc.vector.memset(HI[:], HI0)
        nc.vector.memset(CHI[:], CHI0)

        for rnd, (tgt, eng) in enumerate(zip(TARGETS, ROUND_ENGINES)):
            # T = LO + round_even((HI-LO)*clip((CLO-tgt)/(CLO-CHI)))
            nc.vector.tensor_tensor(A[:], CLO[:], CHI[:], AO.subtract)
            nc.vector.reciprocal(RPc[:], A[:])
            nc.vector.tensor_scalar(A[:], CLO[:], float(tgt), None, AO.subtract)
            nc.vector.tensor_tensor(A[:], A[:], RPc[:], AO.mult)
            nc.vector.tensor_scalar(A[:], A[:], 0.02, 0.98, AO.max, AO.min)
            nc.vector.tensor_tensor(T[:], HI[:], LO[:], AO.subtract)
            nc.vector.tensor_tensor(A[:], T[:], A[:], AO.mult)
            nc.vector.tensor_scalar(A[:], A[:], P23, MAGIC, AO.mult, AO.add)
            nc.vector.tensor_scalar(A[:], A[:], MAGIC, None, AO.subtract)
            nc.vector.tensor_scalar(A[:], A[:], IP23, None, AO.mult)
            nc.vector.tensor_tensor(T[:], LO[:], A[:], AO.add)

            if eng == "act":
                # NT on ACT so probe activations need no cross-engine wait
                nc.scalar.mul(NT[:], T[:], -1.0)
                for i in range(Gg):
                    sgn = scr.tile([P, D], dt, tag="sgn", name="sgn")
                    nc.scalar.activation(
                        sgn[:],
                        r_t[i][:],
                        AF.Sign,
                        bias=NT[:, i : i + 1],
                        scale=1.0,
                        accum_out=SR[:, i : i + 1],
                    )
                nc.vector.tensor_scalar(C[:], SR[:], 2048.0, 0.5, AO.add, AO.mult)
            else:
                for i in range(Gg):
                    sgn = scr.tile([P, D], dt, tag="sgn", name="sgn")
                    nc.vector.tensor_scalar(
                        sgn[:],
                        r_t[i][:],
                        T[:, i : i + 1],
                        None,
                        AO.is_gt,
                        AO.add,
                        accum_out=C[:, i : i + 1],
                    )

            nc.vector.tensor_scalar(UP[:], C[:], 512.0, None, AO.is_ge)
            nc.vector.tensor_scalar(DN[:], C[:], 511.0, None, AO.is_le)
            for dst, src, sel in (
                (LO, T, UP),
                (CLO, C, UP),
                (HI, T, DN),
                (CHI, C, DN),
            ):
                nc.vector.tensor_tensor(A[:], src[:], dst[:], AO.subtract)
                nc.vector.tensor_tensor(A[:], A[:], sel[:], AO.mult)
                nc.vector.tensor_tensor(dst[:], dst[:], A[:], AO.add)

        # m = clip(512 - CHI, 1, 16)
        nc.vector.tensor_scalar(M[:], CHI[:], -1.0, 512.0, AO.mult, AO.add)
        nc.vector.tensor_scalar(M[:], M[:], 1.0, 16.0, AO.max, AO.min)

        # ---- apply phase ----
        for i, t in enumerate(tiles):
            row = t * P
            dtile = dp.tile([P, D], dt, tag="d", name="dtl")
            ntile = np_.tile([P, D], dt, tag="n", name="ntl")
            nc.sync.dma_start(dtile[:], d_d[row : row + P, :])
            nc.sync.dma_start(ntile[:], n_d[row : row + P, :])
            # noisy = data + 0.1*noise
            eng = nc.gpsimd if USE_GPSIMD else nc.vector
            nc.scalar.activation(ntile[:], ntile[:], AF.Copy, bias=0.0, scale=NOISE_STD)
            eng.tensor_tensor(dtile[:], dtile[:], ntile[:], AO.add)

            # w = r * (r <= hi)
            w01 = wp01.tile([P, D], dt, tag="w01", name="w01")
            eng.tensor_scalar(
                w01[:], r_t[i][:], HI[:, i : i + 1], None, AO.is_le
            )
            w = wp.tile([P, D], dt, tag="w", name="w")
            eng.tensor_tensor(w[:], r_t[i][:], w01[:], AO.mult)

            # top16 of w
            t16 = smp.tile([P, 16], dt, tag="t16", name="t16")
            nc.vector.max(t16[:, 0:8], w[:])
            w2 = scr.tile([P, D], dt, tag="w2", name="w2")
            nc.vector.match_replace(w2[:], t16[:, 0:8], w[:], 0.0)
            nc.vector.max(t16[:, 8:16], w2[:])

            # t* = t16[m-1] : onehot(iota16 == m) dot t16
            oh = smp.tile([P, 16], dt, tag="oh", name="oh")
            nc.vector.tensor_scalar(
                oh[:], iota16[:], M[:, i : i + 1], None, AO.is_equal
            )
            nc.vector.tensor_tensor(oh[:], oh[:], t16[:], AO.mult)
            nc.vector.tensor_reduce(TST[:, i : i + 1], oh[:], AX.X, AO.add)

            # mask_inverse = (r < t*)
            mi = mip.tile([P, D], dt, tag="mi", name="mi")
            eng.tensor_scalar(
                mi[:], r_t[i][:], TST[:, i : i + 1], None, AO.is_lt
            )
            # masked = noisy * mask_inverse
            nc.vector.tensor_tensor(dtile[:], dtile[:], mi[:], AO.mult)

            nc.sync.dma_start(om_d[row : row + P, :], dtile[:])
            nc.sync.dma_start(omi_d[row : row + P, :], mi[:])


def build_program(n_tiles=N_TILES, group=GROUP, loop_n=1):
    """Build the SPMD bass program (one core's view)."""
    from contextlib import ExitStack

    import concourse.bacc as bacc
    import concourse.tile as tile
    from concourse import mybir

    rows = n_tiles * P
    nc = bacc.Bacc(None, debug=False)
    dt = mybir.dt.float32
    r_d = nc.dram_tensor("rand", [rows, D], dt, kind="ExternalInput")
    d_d = nc.dram_tensor("data", [rows, D], dt, kind="ExternalInput")
    n_d = nc.dram_tensor("noise", [rows, D], dt, kind="ExternalInput")
    om_d = nc.dram_tensor("masked", [rows, D], dt, kind="ExternalOutput")
    omi_d = nc.dram_tensor("maskinv", [rows, D], dt, kind="ExternalOutput")
    with tile.TileContext(nc) as tc, ExitStack() as ctx:
        emit(tc, nc, r_d, d_d, n_d, om_d, omi_d, n_tiles, group, ctx, loop_n=loop_n)
    return nc


def _tie_patch(r, mask_inv, masked, noisy_fn):
    """Exactly fix rows where the 512th value is tied at the boundary.
    jax top_k keeps the lowest-index elements among tied values."""
    rowsum = mask_inv.sum(axis=1)
    bad = np.where(rowsum != np.float32(D - K))[0]
    for row in bad:
        rr = r[row]
        mask = mask_inv[row] == 0.0
        if not mask.any():
            continue
        tstar = rr[mask].min()
        n_gt = int((rr > tstar).sum())
        need = K - n_gt
        tie_idx = np.where(rr == tstar)[0]
        if need < 0 or need > len(tie_idx):
            continue  # not a tie artifact; leave for the caller's check
        keep = tie_idx[need:]
        if len(keep):
            nz = noisy_fn(row)
            mask_inv[row, keep] = 1.0
            masked[row, keep] = nz[keep]
    return mask_inv, masked


def kernel(data, noise, rand_vals):
    from concourse.bass_utils import run_bass_kernel_spmd

    if "nc" not in _CACHE:
        nc = build_program()
        if not nc.is_finalized():
            nc.finalize()
        _CACHE["nc"] = nc
    nc = _CACHE["nc"]

    d2 = np.ascontiguousarray(data.reshape(ROWS_TOTAL, D), dtype=np.float32)
    n2 = np.ascontiguousarray(noise.reshape(ROWS_TOTAL, D), dtype=np.float32)
    r2 = np.ascontiguousarray(rand_vals.reshape(ROWS_TOTAL, D), dtype=np.float32)

    in_maps = []
    for c in range(N_CORES):
        s = slice(c * ROWS_PER_CORE, (c + 1) * ROWS_PER_CORE)
        in_maps.append(
            {
                "rand": np.ascontiguousarray(r2[s]),
                "data": np.ascontiguousarray(d2[s]),
                "noise": np.ascontiguousarray(n2[s]),
            }
        )

    res = run_bass_kernel_spmd(nc, in_maps, list(range(N_CORES)))
    _CACHE["last_results"] = res
    masked = np.concatenate([res.results[c]["masked"] for c in range(N_CORES)], axis=0)
    mask_inv = np.concatenate(
        [res.results[c]["maskinv"] for c in range(N_CORES)], axis=0
    )

    def noisy_fn(row):
        return (d2[row] + np.float32(NOISE_STD) * n2[row]).astype(np.float32)

    mask_inv, masked = _tie_patch(r2, mask_inv, masked, noisy_fn)

    return masked.reshape(B_SHAPE), mask_inv.reshape(B_SHAPE)


# revision 16
# speedup vs baseline: 6.3596x; 4.0700x over previous
"""Trainium2 Bass kernel for nn_BaseObservationModel (topk masking).

Computes, for x = (32,1024,2048) inputs flattened to rows of D=2048:
    noisy    = data + 0.1*noise
    mask     = positions of the 512 largest rand_vals per row
    masked   = noisy * (1-mask);  mask_inverse = (1-mask) as f32

Device algorithm (per row, exact):
  rand_vals are j*2^-23 (jax uniform) -> probe thresholds on the odd 2^-24
  grid never collide with data. Regula-falsi bracket search (6 counting
  probes, counts via ACT Sign+accum / DVE is_gt+accum) finds hi with
  c_hi = #{r > hi} in [496, 511]. Then w = r*(r<=hi), top-16 of w via
  DVE Max8 + MatchReplace + Max8, and t* = the (512-c_hi)-th largest of w
  == the 512th largest of the row. mask_inverse = (r < t*).
  Rows where the 512th value is tied at the boundary (2 rows in this
  dataset) are patched exactly on the host from the device output.

Data parallel: 32768 rows sharded 4096/core across 8 cores.
"""

import numpy as np

# ---------------- hardcoded problem config ----------------
B_SHAPE = (32, 1024, 2048)
D = 2048
K = 512
N_CORES = 8
ROWS_TOTAL = 32768
ROWS_PER_CORE = ROWS_TOTAL // N_CORES  # 4096
P = 128
N_TILES = ROWS_PER_CORE // P  # 32
GROUP = 4  # tiles per probe-batch group
TARGETS = [512.0, 512.0, 508.0, 507.0, 506.0, 505.0]
# engine per probe round: 'act' = Sign+accum on ScalarE, 'dve' = is_gt on VectorE
ROUND_ENGINES = ["act", "act", "act", "act", "act", "dve"]

LO0 = 3.0 / 16777216.0
CLO0 = 2048.0
HI0 = 16777215.0 / 16777216.0
CHI0 = 0.0
MAGIC = 12582912.0
P23 = 8388608.0
IP23 = 1.0 / 8388608.0
NOISE_STD = 0.1

USE_GPSIMD = False  # GPSIMD elementwise measured ~2x slower than modeled; keep on DVE

_CACHE = {}


def emit(tc, nc, r_d, d_d, n_d, om_d, omi_d, n_tiles, group, ctx, loop_n=1):
    """Emit the tile program. r_d/d_d/n_d inputs, om_d/omi_d outputs: DRAM
    tensors of [n_tiles*128, 2048] f32."""
    from concourse import mybir
    from concourse.alu_op_type import AluOpType as AO

    dt = mybir.dt.float32
    AF = mybir.ActivationFunctionType
    AX = mybir.AxisListType

    G = group
    n_groups = (n_tiles + G - 1) // G

    rp = ctx.enter_context(tc.tile_pool(name="rp", bufs=2))
    dp = ctx.enter_context(tc.tile_pool(name="dp", bufs=3))
    np_ = ctx.enter_context(tc.tile_pool(name="np", bufs=3))
    scr = ctx.enter_context(tc.tile_pool(name="scr", bufs=2))
    wp01 = ctx.enter_context(tc.tile_pool(name="wp01", bufs=2))
    wp = ctx.enter_context(tc.tile_pool(name="wp", bufs=2))
    mip = ctx.enter_context(tc.tile_pool(name="mip", bufs=2))
    smp = ctx.enter_context(tc.tile_pool(name="smp", bufs=2))  # small per-group state
    cst = ctx.enter_context(tc.tile_pool(name="cst", bufs=1))

    # constants
    iota16 = cst.tile([P, 16], dt, tag="iota16", name="iota16")
    nc.gpsimd.iota(
        iota16[:],
        pattern=[[1, 16]],
        base=1,
        channel_multiplier=0,
        allow_small_or_imprecise_dtypes=True,
    )

    for _rep in range(loop_n):
        _emit_groups(tc, nc, r_d, d_d, n_d, om_d, omi_d, n_tiles, G, n_groups,
                     iota16, rp, dp, np_, scr, wp01, wp, mip, smp)


def _emit_groups(tc, nc, r_d, d_d, n_d, om_d, omi_d, n_tiles, G, n_groups,
                 iota16, rp, dp, np_, scr, wp01, wp, mip, smp):
    from concourse import mybir
    from concourse.alu_op_type import AluOpType as AO

    dt = mybir.dt.float32
    AF = mybir.ActivationFunctionType
    AX = mybir.AxisListType

    for g in range(n_groups):
        tiles = [g * G + i for i in range(G) if g * G + i < n_tiles]
        Gg = len(tiles)

        # ---- load rand tiles for the group ----
        r_t = []
        for i, t in enumerate(tiles):
            rt = rp.tile([P, D], dt, tag=f"r{i}", name=f"r{i}")
            nc.sync.dma_start(rt[:], r_d[t * P : (t + 1) * P, :])
            r_t.append(rt)

        # ---- per-group state [P, Gg] ----
        def st(tag):
            return smp.tile([P, Gg], dt, tag=tag, name=tag)

        LO, CLO, HI, CHI = st("LO"), st("CLO"), st("HI"), st("CHI")
        T, NT, SR, C = st("T"), st("NT"), st("SR"), st("C")
        UP, DN, A, RPc = st("UP"), st("DN"), st("A"), st("RP")
        M, TST = st("M"), st("TST")

        nc.vector.memset(LO[:], LO0)
        nc.vector.memset(CLO[:], CLO0)
        nc.vector.memset(HI[:], HI0)
        nc.vector.memset(CHI[:], CHI0)

        for rnd, (tgt, eng) in enumerate(zip(TARGETS, ROUND_ENGINES)):
            # T = LO + round_even((HI-LO)*clip((CLO-tgt)/(CLO-CHI)))
            nc.vector.tensor_tensor(A[:], CLO[:], CHI[:], AO.subtract)
            nc.vector.reciprocal(RPc[:], A[:])
            nc.vector.tensor_scalar(A[:], CLO[:], float(tgt), None, AO.subtract)
            nc.vector.tensor_tensor(A[:], A[:], RPc[:], AO.mult)
            nc.vector.tensor_scalar(A[:], A[:], 0.02, 0.98, AO.max, AO.min)
            nc.vector.tensor_tensor(T[:], HI[:], LO[:], AO.subtract)
            nc.vector.tensor_tensor(A[:], T[:], A[:], AO.mult)
            nc.vector.tensor_scalar(A[:], A[:], P23, MAGIC, AO.mult, AO.add)
            nc.vector.tensor_scalar(A[:], A[:], MAGIC, None, AO.subtract)
            nc.vector.tensor_scalar(A[:], A[:], IP23, None, AO.mult)
            nc.vector.tensor_tensor(T[:], LO[:], A[:], AO.add)

            if eng == "act":
                # NT on ACT so probe activations need no cross-engine wait
                nc.scalar.mul(NT[:], T[:], -1.0)
                for i in range(Gg):
                    sgn = scr.tile([P, D], dt, tag="sgn", name="sgn")
                    nc.scalar.activation(
                        sgn[:],
                        r_t[i][:],
                        AF.Sign,
                        bias=NT[:, i : i + 1],
                        scale=1.0,
                        accum_out=SR[:, i : i + 1],
                    )
                nc.vector.tensor_scalar(C[:], SR[:], 2048.0, 0.5, AO.add, AO.mult)
            else:
                for i in range(Gg):
                    sgn = scr.tile([P, D], dt, tag="sgn", name="sgn")
                    nc.vector.tensor_scalar(
                        sgn[:],
                        r_t[i][:],
                        T[:, i : i + 1],
                        None,
                        AO.is_gt,
                        AO.add,
                        accum_out=C[:, i : i + 1],
                    )

            nc.vector.tensor_scalar(UP[:], C[:], 512.0, None, AO.is_ge)
            nc.vector.tensor_scalar(DN[:], C[:], 511.0, None, AO.is_le)
            for dst, src, sel in (
                (LO, T, UP),
                (CLO, C, UP),
                (HI, T, DN),
                (CHI, C, DN),
            ):
                nc.vector.tensor_tensor(A[:], src[:], dst[:], AO.subtract)
                nc.vector.tensor_tensor(A[:], A[:], sel[:], AO.mult)
                nc.vector.tensor_tensor(dst[:], dst[:], A[:], AO.add)

        # m = clip(512 - CHI, 1, 16)
        nc.vector.tensor_scalar(M[:], CHI[:], -1.0, 512.0, AO.mult, AO.add)
        nc.vector.tensor_scalar(M[:], M[:], 1.0, 16.0, AO.max, AO.min)

        # ---- apply phase ----
        for i, t in enumerate(tiles):
            row = t * P
            dtile = dp.tile([P, D], dt, tag="d", name="dtl")
            ntile = np_.tile([P, D], dt, tag="n", name="ntl")
            nc.sync.dma_start(dtile[:], d_d[row : row + P, :])
            nc.sync.dma_start(ntile[:], n_d[row : row + P, :])
            # noisy = data + 0.1*noise
            eng = nc.gpsimd if USE_GPSIMD else nc.vector
            nc.scalar.activation(ntile[:], ntile[:], AF.Copy, bias=0.0, scale=NOISE_STD)
            eng.tensor_tensor(dtile[:], dtile[:], ntile[:], AO.add)

            # w = r * (r <= hi)
            w01 = wp01.tile([P, D], dt, tag="w01", name="w01")
            eng.tensor_scalar(
                w01[:], r_t[i][:], HI[:, i : i + 1], None, AO.is_le
            )
            w = wp.tile([P, D], dt, tag="w", name="w")
            eng.tensor_tensor(w[:], r_t[i][:], w01[:], AO.mult)

            # top16 of w
            t16 = smp.tile([P, 16], dt, tag="t16", name="t16")
            nc.vector.max(t16[:, 0:8], w[:])
            w2 = scr.tile([P, D], dt, tag="w2", name="w2")
            nc.vector.match_replace(w2[:], t16[:, 0:8], w[:], 0.0)
            nc.vector.max(t16[:, 8:16], w2[:])

            # t* = t16[m-1] : onehot(iota16 == m) dot t16
            oh = smp.tile([P, 16], dt, tag="oh", name="oh")
            nc.vector.tensor_scalar(
                oh[:], iota16[:], M[:, i : i + 1], None, AO.is_equal
            )
            nc.vector.tensor_tensor(oh[:], oh[:], t16[:], AO.mult)
            nc.vector.tensor_reduce(TST[:, i : i + 1], oh[:], AX.X, AO.add)

            # mask_inverse = (r < t*)
            mi = mip.tile([P, D], dt, tag="mi", name="mi")
            eng.tensor_scalar(
                mi[:], r_t[i][:], TST[:, i : i + 1], None, AO.is_lt
            )
            # masked = noisy * mask_inverse
            nc.vector.tensor_tensor(dtile[:], dtile[:], mi[:], AO.mult)

            nc.sync.dma_start(om_d[row : row + P, :], dtile[:])
            nc.sync.dma_start(omi_d[row : row + P, :], mi[:])


def build_program(n_tiles=N_TILES, group=GROUP, loop_n=1):
    """Build the SPMD bass program (one core's view)."""
    from contextlib import ExitStack

    import concourse.bacc as bacc
    import concourse.tile as tile
    from concourse import mybir

    rows = n_tiles * P
    nc = bacc.Bacc(None, debug=False)
    dt = mybir.dt.float32
    r_d = nc.dram_tensor("rand", [rows, D], dt, kind="ExternalInput")
    d_d = nc.dram_tensor("data", [rows, D], dt, kind="ExternalInput")
    n_d = nc.dram_tensor("noise", [rows, D], dt, kind="ExternalInput")
    om_d = nc.dram_tensor("masked", [rows, D], dt, kind="ExternalOutput")
    omi_d = nc.dram_tensor("maskinv", [rows, D], dt, kind="ExternalOutput")
    with tile.TileContext(nc) as tc, ExitStack() as ctx:
        emit(tc, nc, r_d, d_d, n_d, om_d, omi_d, n_tiles, group, ctx, loop_n=loop_n)
    return nc


def _tie_patch(r, mask_inv, masked, noisy_fn):
    """Exactly fix rows where the 512th value is tied at the boundary.
    jax top_k keeps the lowest-index elements among tied values."""
    rowsum = mask_inv.sum(axis=1)
    bad = np.where(rowsum != np.float32(D - K))[0]
    for row in bad:
        rr = r[row]
        mask = mask_inv[row] == 0.0
        if not mask.any():
            continue
        tstar = rr[mask].min()
        n_gt = int((rr > tstar).sum())
        need = K - n_gt
        tie_idx = np.where(rr == tstar)[0]
        if need < 0 or need > len(tie_idx):
            continue  # not a tie artifact; leave for the caller's check
        keep = tie_idx[need:]
        if len(keep):
            nz = noisy_fn(row)
            mask_inv[row, keep] = 1.0
            masked[row, keep] = nz[keep]
    return mask_inv, masked


def kernel(data, noise, rand_vals):
    from concourse.bass_utils import run_bass_kernel_spmd

    if "nc" not in _CACHE:
        nc = build_program()
        if not nc.is_finalized():
            nc.finalize()
        _CACHE["nc"] = nc
    nc = _CACHE["nc"]

    d2 = np.ascontiguousarray(data.reshape(ROWS_TOTAL, D), dtype=np.float32)
    n2 = np.ascontiguousarray(noise.reshape(ROWS_TOTAL, D), dtype=np.float32)
    r2 = np.ascontiguousarray(rand_vals.reshape(ROWS_TOTAL, D), dtype=np.float32)

    in_maps = []
    for c in range(N_CORES):
        s = slice(c * ROWS_PER_CORE, (c + 1) * ROWS_PER_CORE)
        in_maps.append(
            {
                "rand": np.ascontiguousarray(r2[s]),
                "data": np.ascontiguousarray(d2[s]),
                "noise": np.ascontiguousarray(n2[s]),
            }
        )

    res = run_bass_kernel_spmd(nc, in_maps, list(range(N_CORES)))
    _CACHE["last_results"] = res
    masked = np.concatenate([res.results[c]["masked"] for c in range(N_CORES)], axis=0)
    mask_inv = np.concatenate(
        [res.results[c]["maskinv"] for c in range(N_CORES)], axis=0
    )

    def noisy_fn(row):
        return (d2[row] + np.float32(NOISE_STD) * n2[row]).astype(np.float32)

    mask_inv, masked = _tie_patch(r2, mask_inv, masked, noisy_fn)

    return masked.reshape(B_SHAPE), mask_inv.reshape(B_SHAPE)


# revision 19
# speedup vs baseline: 10.3790x; 1.6320x over previous
"""Trainium2 Bass kernel for nn_BaseObservationModel (topk masking).

Computes, for x = (32,1024,2048) inputs flattened to rows of D=2048:
    noisy    = data + 0.1*noise
    mask     = positions of the 512 largest rand_vals per row
    masked   = noisy * (1-mask);  mask_inverse = (1-mask) as f32

Device algorithm (per row, exact):
  rand_vals are j*2^-23 (jax uniform) -> probe thresholds on the odd 2^-24
  grid never collide with data. Regula-falsi bracket search (6 counting
  probes, counts via ACT Sign+accum / DVE is_gt+accum) finds hi with
  c_hi = #{r > hi} in [496, 511]. Then w = r*(r<=hi), top-16 of w via
  DVE Max8 + MatchReplace + Max8, and t* = the (512-c_hi)-th largest of w
  == the 512th largest of the row. mask_inverse = (r < t*).
  Rows where the 512th value is tied at the boundary (2 rows in this
  dataset) are patched exactly on the host from the device output.

Data parallel: 32768 rows sharded 4096/core across 8 cores.
"""

import numpy as np

# ---------------- hardcoded problem config ----------------
B_SHAPE = (32, 1024, 2048)
D = 2048
K = 512
N_CORES = 8
ROWS_TOTAL = 32768
ROWS_PER_CORE = ROWS_TOTAL // N_CORES  # 4096
P = 128
N_TILES = ROWS_PER_CORE // P  # 32
GROUP = 6  # tiles per probe-batch group
TARGETS = [512.0, 512.0, 508.0, 507.0, 506.0, 505.0]
# engine per probe round: 'act' = Sign+accum on ScalarE, 'dve' = is_gt on VectorE
ROUND_ENGINES = ["act", "act", "act", "act", "act", "dve"]

LO0 = 3.0 / 16777216.0
CLO0 = 2048.0
HI0 = 16777215.0 / 16777216.0
CHI0 = 0.0
MAGIC = 12582912.0
P23 = 8388608.0
IP23 = 1.0 / 8388608.0
NOISE_STD = 0.1

USE_GPSIMD = False  # GPSIMD elementwise measured ~2x slower than modeled; keep on DVE

_CACHE = {}


def emit(tc, nc, r_d, d_d, n_d, om_d, omi_d, n_tiles, group, ctx, loop_n=1):
    """Emit the tile program. r_d/d_d/n_d inputs, om_d/omi_d outputs: DRAM
    tensors of [n_tiles*128, 2048] f32."""
    from concourse import mybir
    from concourse.alu_op_type import AluOpType as AO

    dt = mybir.dt.float32
    AF = mybir.ActivationFunctionType
    AX = mybir.AxisListType

    G = group
    n_groups = (n_tiles + G - 1) // G

    rp = ctx.enter_context(tc.tile_pool(name="rp", bufs=2))
    dp = ctx.enter_context(tc.tile_pool(name="dp", bufs=2))
    np_ = ctx.enter_context(tc.tile_pool(name="np", bufs=2))
    scr = ctx.enter_context(tc.tile_pool(name="scr", bufs=2))
    wp = ctx.enter_context(tc.tile_pool(name="wp", bufs=2))
    mip = ctx.enter_context(tc.tile_pool(name="mip", bufs=2))
    smp = ctx.enter_context(tc.tile_pool(name="smp", bufs=2))  # small per-group state
    cst = ctx.enter_context(tc.tile_pool(name="cst", bufs=1))

    # constants
    iota16 = cst.tile([P, 16], dt, tag="iota16", name="iota16")
    nc.gpsimd.iota(
        iota16[:],
        pattern=[[1, 16]],
        base=1,
        channel_multiplier=0,
        allow_small_or_imprecise_dtypes=True,
    )

    for _rep in range(loop_n):
        _emit_groups(tc, nc, r_d, d_d, n_d, om_d, omi_d, n_tiles, G, n_groups,
                     iota16, rp, dp, np_, scr, wp, mip, smp)


def _emit_groups(tc, nc, r_d, d_d, n_d, om_d, omi_d, n_tiles, G, n_groups,
                 iota16, rp, dp, np_, scr, wp, mip, smp):
    from concourse import mybir
    from concourse.alu_op_type import AluOpType as AO

    dt = mybir.dt.float32
    AF = mybir.ActivationFunctionType
    AX = mybir.AxisListType

    for g in range(n_groups):
        tiles = [g * G + i for i in range(G) if g * G + i < n_tiles]
        Gg = len(tiles)

        # ---- load rand tiles for the group ----
        r_t = []
        for i, t in enumerate(tiles):
            rt = rp.tile([P, D], dt, tag=f"r{i}", name=f"r{i}")
            nc.sync.dma_start(rt[:], r_d[t * P : (t + 1) * P, :])
            r_t.append(rt)

        # ---- per-group state [P, Gg] ----
        def st(tag):
            return smp.tile([P, Gg], dt, tag=tag, name=tag)

        ST4 = smp.tile([P, 4 * Gg], dt, tag="ST4", name="ST4")
        LO, CLO = ST4[:, 0:Gg], ST4[:, Gg : 2 * Gg]
        HI, CHI = ST4[:, 2 * Gg : 3 * Gg], ST4[:, 3 * Gg : 4 * Gg]
        T, NT, SR, C = st("T"), st("NT"), st("SR"), st("C")
        UP, DN, A, RPc = st("UP"), st("DN"), st("A"), st("RP")
        M, TST = st("M"), st("TST")

        nc.vector.memset(LO, LO0)
        nc.vector.memset(CLO, CLO0)
        nc.vector.memset(HI, HI0)
        nc.vector.memset(CHI, CHI0)

        for rnd, (tgt, eng) in enumerate(zip(TARGETS, ROUND_ENGINES)):
            # T = LO + round_even((HI-LO)*clip((CLO-tgt)/(CLO-CHI)))
            nc.vector.tensor_tensor(A[:], CLO, CHI, AO.subtract)
            nc.vector.reciprocal(RPc[:], A[:])
            nc.vector.tensor_scalar(DN[:], CLO, float(tgt), None, AO.subtract)
            nc.vector.tensor_tensor(A[:], DN[:], RPc[:], AO.mult)
            nc.vector.tensor_scalar(A[:], A[:], 0.02, 0.98, AO.max, AO.min)
            nc.vector.tensor_tensor(T[:], HI, LO, AO.subtract)
            nc.vector.tensor_tensor(A[:], T[:], A[:], AO.mult)
            nc.vector.tensor_scalar(A[:], A[:], P23, MAGIC, AO.mult, AO.add)
            nc.vector.tensor_scalar(A[:], A[:], MAGIC, IP23, AO.subtract, AO.mult)
            nc.vector.tensor_tensor(T[:], LO, A[:], AO.add)

            if eng == "act":
                # NT on ACT so probe activations need no cross-engine wait
                nc.scalar.mul(NT[:], T[:], -1.0)
                for i in range(Gg):
                    sgn = scr.tile([P, D], dt, tag="sgn", name="sgn")
                    nc.scalar.activation(
                        sgn[:],
                        r_t[i][:],
                        AF.Sign,
                        bias=NT[:, i : i + 1],
                        scale=1.0,
                        accum_out=SR[:, i : i + 1],
                    )
                nc.vector.tensor_scalar(C[:], SR[:], 2048.0, 0.5, AO.add, AO.mult)
            else:
                for i in range(Gg):
                    sgn = scr.tile([P, D], dt, tag="sgn", name="sgn")
                    nc.vector.tensor_scalar(
                        sgn[:],
                        r_t[i][:],
                        T[:, i : i + 1],
                        None,
                        AO.is_gt,
                        AO.add,
                        accum_out=C[:, i : i + 1],
                    )

            nc.vector.tensor_scalar(UP[:], C[:], 512.0, None, AO.is_ge)
            nc.vector.tensor_scalar(DN[:], C[:], 511.0, None, AO.is_le)
            for dst, srcv, sel in (
                (LO, T[:], UP),
                (CLO, C[:], UP),
                (HI, T[:], DN),
                (CHI, C[:], DN),
            ):
                nc.vector.tensor_tensor(A[:], srcv, dst, AO.subtract)
                nc.vector.tensor_tensor(A[:], A[:], sel[:], AO.mult)
                nc.vector.tensor_tensor(dst, dst, A[:], AO.add)

        # m = clip(512 - CHI, 1, 16)
        nc.vector.tensor_scalar(M[:], CHI, -1.0, 512.0, AO.mult, AO.add)
        nc.vector.tensor_scalar(M[:], M[:], 1.0, 16.0, AO.max, AO.min)

        # ---- apply phase ----
        for i, t in enumerate(tiles):
            row = t * P
            dtile = dp.tile([P, D], dt, tag="d", name="dtl")
            ntile = np_.tile([P, D], dt, tag="n", name="ntl")
            nc.sync.dma_start(dtile[:], d_d[row : row + P, :])
            nc.sync.dma_start(ntile[:], n_d[row : row + P, :])
            # noisy = 0.1*noise + data (one fused DVE op)
            nc.vector.scalar_tensor_tensor(
                dtile[:], ntile[:], NOISE_STD, dtile[:], AO.mult, AO.add
            )

            # w = (r <= hi) * r (one fused DVE op)
            w = wp.tile([P, D], dt, tag="w", name="w")
            nc.vector.scalar_tensor_tensor(
                w[:], r_t[i][:], HI[:, i : i + 1], r_t[i][:], AO.is_le, AO.mult
            )

            # top16 of w
            t16 = smp.tile([P, 16], dt, tag="t16", name="t16")
            nc.vector.max(t16[:, 0:8], w[:])
            w2 = scr.tile([P, D], dt, tag="w2", name="w2")
            nc.vector.match_replace(w2[:], t16[:, 0:8], w[:], 0.0)
            nc.vector.max(t16[:, 8:16], w2[:])

            # t* = t16[m-1] : fused (iota16==m)*t16 with accumulate
            oh = smp.tile([P, 16], dt, tag="oh", name="oh")
            nc.vector.scalar_tensor_tensor(
                oh[:], iota16[:], M[:, i : i + 1], t16[:],
                AO.is_equal, AO.mult, accum_out=TST[:, i : i + 1],
            )

            # mask_inverse = (r < t*)
            mi = mip.tile([P, D], dt, tag="mi", name="mi")
            nc.vector.tensor_scalar(
                mi[:], r_t[i][:], TST[:, i : i + 1], None, AO.is_lt
            )
            # masked = noisy * mask_inverse
            nc.vector.tensor_tensor(dtile[:], dtile[:], mi[:], AO.mult)

            nc.sync.dma_start(om_d[row : row + P, :], dtile[:])
            nc.sync.dma_start(omi_d[row : row + P, :], mi[:])


def build_program(n_tiles=N_TILES, group=GROUP, loop_n=1):
    """Build the SPMD bass program (one core's view)."""
    from contextlib import ExitStack

    import concourse.bacc as bacc
    import concourse.tile as tile
    from concourse import mybir

    rows = n_tiles * P
    nc = bacc.Bacc(None, debug=False)
    dt = mybir.dt.float32
    r_d = nc.dram_tensor("rand", [rows, D], dt, kind="ExternalInput")
    d_d = nc.dram_tensor("data", [rows, D], dt, kind="ExternalInput")
    n_d = nc.dram_tensor("noise", [rows, D], dt, kind="ExternalInput")
    om_d = nc.dram_tensor("masked", [rows, D], dt, kind="ExternalOutput")
    omi_d = nc.dram_tensor("maskinv", [rows, D], dt, kind="ExternalOutput")
    with tile.TileContext(nc) as tc, ExitStack() as ctx:
        emit(tc, nc, r_d, d_d, n_d, om_d, omi_d, n_tiles, group, ctx, loop_n=loop_n)
    return nc


def _tie_patch(r, mask_inv, masked, noisy_fn):
    """Exactly fix rows where the 512th value is tied at the boundary.
    jax top_k keeps the lowest-index elements among tied values."""
    rowsum = mask_inv.sum(axis=1)
    bad = np.where(rowsum != np.float32(D - K))[0]
    for row in bad:
        rr = r[row]
        mask = mask_inv[row] == 0.0
        if not mask.any():
            continue
        tstar = rr[mask].min()
        n_gt = int((rr > tstar).sum())
        need = K - n_gt
        tie_idx = np.where(rr == tstar)[0]
        if need < 0 or need > len(tie_idx):
            continue  # not a tie artifact; leave for the caller's check
        keep = tie_idx[need:]
        if len(keep):
            nz = noisy_fn(row)
            mask_inv[row, keep] = 1.0
            masked[row, keep] = nz[keep]
    return mask_inv, masked


def kernel(data, noise, rand_vals):
    from concourse.bass_utils import run_bass_kernel_spmd

    if "nc" not in _CACHE:
        nc = build_program()
        if not nc.is_finalized():
            nc.finalize()
        _CACHE["nc"] = nc
    nc = _CACHE["nc"]

    d2 = np.ascontiguousarray(data.reshape(ROWS_TOTAL, D), dtype=np.float32)
    n2 = np.ascontiguousarray(noise.reshape(ROWS_TOTAL, D), dtype=np.float32)
    r2 = np.ascontiguousarray(rand_vals.reshape(ROWS_TOTAL, D), dtype=np.float32)

    in_maps = []
    for c in range(N_CORES):
        s = slice(c * ROWS_PER_CORE, (c + 1) * ROWS_PER_CORE)
        in_maps.append(
            {
                "rand": np.ascontiguousarray(r2[s]),
                "data": np.ascontiguousarray(d2[s]),
                "noise": np.ascontiguousarray(n2[s]),
            }
        )

    res = run_bass_kernel_spmd(nc, in_maps, list(range(N_CORES)))
    _CACHE["last_results"] = res
    masked = np.concatenate([res.results[c]["masked"] for c in range(N_CORES)], axis=0)
    mask_inv = np.concatenate(
        [res.results[c]["maskinv"] for c in range(N_CORES)], axis=0
    )

    def noisy_fn(row):
        return (d2[row] + np.float32(NOISE_STD) * n2[row]).astype(np.float32)

    mask_inv, masked = _tie_patch(r2, mask_inv, masked, noisy_fn)

    return masked.reshape(B_SHAPE), mask_inv.reshape(B_SHAPE)
